# Initial kernel scaffold
#
"""Distribution tokenizer (per-row 64-bin histogram) for Trainium2, 8 NeuronCores.

Problem: x (32, 512, 1024) f32, boundaries (63,) f32 sorted ascending.
For every row (b, t): bin(x) = #{j : boundaries[j] <= x} (searchsorted right),
z[b, t, k] = count of bin k in the 1024-element feature row / 1024.

Algorithm (exact, no approximations):
  For each threshold j: H_j = #{f : x[f] >= b_j}. Then with Hext =
  [F, H_0, ..., H_62, 0], counts[k] = Hext[k] - Hext[k+1], z = counts / 1024
  (division by 2^10 is exact in fp32, counts are integers <= 1024).

  The 63 thresholds are split across two engines working in parallel:
   - DVE (vector): tensor_scalar(op0=is_ge, scalar1=b_j, op1=add reduce,
     accum_out=H_j) -- one fused mask+reduce instruction per threshold.
   - ACT (scalar): pass1 s = Sign(-x + b_j) (+1 iff x < b_j, 0 iff x == b_j),
     pass2 Relu(s) with accum_out = L_j = #{x < b_j}; H_j = F - L_j.
  All comparisons are exact fp32 comparator ops against the exact boundary
  values, so ties (x == b_j) are handled identically to searchsorted.

Sharding: pure data parallel, batch dim 32 -> 8 cores x 4.
"""

import numpy as np

B, T, F = 32, 512, 1024
NB = 64            # number of bins
NTH = NB - 1       # number of thresholds (63)
N_CORES = 8
ROWS_PER_CORE = (B // N_CORES) * T        # 2048
P = 128                                   # SBUF partitions
N_TILES = ROWS_PER_CORE // P              # 16

# Threshold split: j in [0, N_DVE) on the vector engine, rest on scalar engine.
N_DVE = 40

_PROGRAM_CACHE = {}


def _build_program(bvals):
    """Build the per-core Bass program. bvals: list of 63 exact float values."""
    import concourse.bass as bass
    import concourse.mybir as mybir
    import concourse.tile as tile

    f32 = mybir.dt.float32
    bf16 = mybir.dt.bfloat16
    Alu = mybir.AluOpType
    Act = mybir.ActivationFunctionType

    nc = bass.Bass(trn_type="TRN2")
    x_d = nc.dram_tensor("x", [ROWS_PER_CORE, F], f32, kind="ExternalInput")
    z_d = nc.dram_tensor("z", [ROWS_PER_CORE, NB], f32, kind="ExternalOutput")

    n_act = NTH - N_DVE

    with tile.TileContext(nc) as tc:
        with (
            tc.tile_pool(name="xp", bufs=3) as xp,
            tc.tile_pool(name="hp", bufs=2) as hp,
            tc.tile_pool(name="lp", bufs=2) as lp,
            tc.tile_pool(name="sp", bufs=2) as sp,
            tc.tile_pool(name="tv", bufs=2) as tv,
            tc.tile_pool(name="ts", bufs=2) as ts,
            tc.tile_pool(name="zp", bufs=2) as zp,
        ):
            for i in range(N_TILES):
                xt = xp.tile([P, F], f32)
                nc.sync.dma_start(xt[:], x_d[bass.ts(i, P), :])

                hext = hp.tile([P, NB + 1], f32)
                nc.vector.memset(hext[:, 0:1], float(F))
                nc.vector.memset(hext[:, NB:NB + 1], 0.0)

                trash_v = tv.tile([P, F], bf16)
                for j in range(N_DVE):
                    nc.vector.tensor_scalar(
                        trash_v[:], xt[:], bvals[j], None,
                        Alu.is_ge, Alu.add,
                        accum_out=hext[:, 1 + j:2 + j],
                    )

                if n_act:
                    lbuf = lp.tile([P, n_act], f32)
                    for k in range(n_act):
                        j = N_DVE + k
                        sgn = sp.tile([P, F], bf16)
                        nc.scalar.activation(
                            sgn[:], xt[:], Act.Sign, bias=bvals[j], scale=-1.0,
                        )
                        trash_s = ts.tile([P, F], bf16)
                        nc.scalar.activation(
                            trash_s[:], sgn[:], Act.Relu,
                            accum_out=lbuf[:, k:k + 1],
                        )
                    # H_j = F - L_j for the ACT-owned columns.
                    nc.vector.tensor_scalar(
                        hext[:, 1 + N_DVE:1 + NTH], lbuf[:], -1.0, float(F),
                        Alu.mult, Alu.add,
                    )

                zt = zp.tile([P, NB], f32)
                nc.vector.tensor_tensor(
                    zt[:], hext[:, 0:NB], hext[:, 1:NB + 1], Alu.subtract,
                )
                nc.vector.tensor_scalar(
                    zt[:], zt[:], float(2.0 ** -10), None, Alu.mult,
                )
                nc.sync.dma_start(z_d[bass.ts(i, P), :], zt[:])

    return nc


def _get_program(b):
    key = b.tobytes()
    if key not in _PROGRAM_CACHE:
        _PROGRAM_CACHE[key] = _build_program([float(v) for v in b])
    return _PROGRAM_CACHE[key]


def run(x, boundaries, trace=False):
    """Run on hardware; returns (z, BassKernelResults)."""
    from concourse.bass_utils import run_bass_kernel_spmd

    x = np.ascontiguousarray(np.asarray(x), dtype=np.float32)
    b = np.ascontiguousarray(np.asarray(boundaries), dtype=np.float32)
    assert x.shape == (B, T, F) and b.shape == (NTH,)

    nc = _get_program(b)
    bpc = B // N_CORES
    in_maps = [
        {"x": np.ascontiguousarray(x[c * bpc:(c + 1) * bpc].reshape(ROWS_PER_CORE, F))}
        for c in range(N_CORES)
    ]
    res = run_bass_kernel_spmd(nc, in_maps, core_ids=list(range(N_CORES)), trace=trace)
    z = np.stack([res.results[c]["z"].reshape(bpc, T, NB) for c in range(N_CORES)])
    return z.reshape(B, T, NB), res


def kernel(x, boundaries, nr_of_bins):
    assert int(nr_of_bins) == NB
    z, _ = run(x, boundaries)
    return z


# revision 12
# speedup vs baseline: 235.4080x; 235.4080x over previous
"""Distribution tokenizer (per-row 64-bin histogram) for Trainium2, 8 NeuronCores.

Problem: x (32, 512, 1024) f32, boundaries (63,) f32 sorted ascending.
For every row (b, t): bin(x) = #{j : boundaries[j] <= x} (searchsorted right),
z[b, t, k] = count of bin k in the 1024-element feature row / 1024.

Algorithm (exact, no approximations):
  For each threshold j: H_j = #{f : x[f] >= b_j}. Then with Hext =
  [F, H_0, ..., H_62, 0], counts[k] = Hext[k] - Hext[k+1], z = counts / 1024
  (division by 2^10 is exact in fp32, counts are integers <= 1024).

  The 63 thresholds are split across two engines working in parallel:
   - DVE (vector): tensor_scalar(op0=is_ge, scalar1=b_j, op1=add reduce,
     accum_out=H_j) -- one fused mask+reduce instruction per threshold.
   - ACT (scalar): pass1 s = Sign(-x + b_j) (+1 iff x < b_j, 0 iff x == b_j),
     pass2 Relu(s) with accum_out = L_j = #{x < b_j}; H_j = F - L_j.
  All comparisons are exact fp32 comparator ops against the exact boundary
  values, so ties (x == b_j) are handled identically to searchsorted.

Sharding: pure data parallel, batch dim 32 -> 8 cores x 4.
"""

import numpy as np

B, T, F = 32, 512, 1024
NB = 64            # number of bins
NTH = NB - 1       # number of thresholds (63)
N_CORES = 8
ROWS_PER_CORE = (B // N_CORES) * T        # 2048
P = 128                                   # SBUF partitions
N_TILES = ROWS_PER_CORE // P              # 16

# Threshold split: j in [0, N_DVE) on the vector engine, rest on scalar engine.
N_DVE = 40

_PROGRAM_CACHE = {}


def _build_program(bvals, repeat=1):
    """Build the per-core Bass program. bvals: list of 63 exact float values.

    repeat>1 re-runs the whole tile loop (perf slope measurement only).
    """
    import concourse.bass as bass
    import concourse.mybir as mybir
    import concourse.tile as tile
    from concourse import bacc

    f32 = mybir.dt.float32
    bf16 = mybir.dt.bfloat16
    Alu = mybir.AluOpType
    Act = mybir.ActivationFunctionType

    # Bacc (not raw Bass): its compile() runs generate_event_semaphores,
    # which splits multi-wait instructions to satisfy the TRN2 limit of
    # one sync wait per instruction.
    nc = bacc.Bacc("TRN2")
    x_d = nc.dram_tensor("x", [ROWS_PER_CORE, F], f32, kind="ExternalInput")
    z_d = nc.dram_tensor("z", [ROWS_PER_CORE, NB], f32, kind="ExternalOutput")

    n_act = NTH - N_DVE

    # Register const [P,1] APs for ACT bias values (boundaries used on ACT and
    # the row total F), exactly like Bass.__init__ does for 0.0/1.0. These are
    # written before the TileContext so tile scheduling sees them as plain
    # constant reads with no tracked writers.
    def register_const(value):
        key = (f32, value)
        if key not in nc.const_aps.aps:
            t = nc.alloc_sbuf_tensor(f"const-f32-{value}", [P, 1], f32)
            nc.gpsimd.memset(t.ap(), value)
            nc.const_aps.aps[key] = t.ap()

    for j in range(N_DVE, NTH):
        register_const(bvals[j])
    register_const(float(F))
    nc.all_engine_barrier()

    with tile.TileContext(nc) as tc:
        with (
            tc.tile_pool(name="xp", bufs=3) as xp,
            tc.tile_pool(name="hp", bufs=2) as hp,
            tc.tile_pool(name="lp", bufs=2) as lp,
            tc.tile_pool(name="hp2", bufs=2) as hp2,
            tc.tile_pool(name="sp", bufs=2) as sp,
            tc.tile_pool(name="tv", bufs=2) as tv,
            tc.tile_pool(name="ts", bufs=2) as ts,
            tc.tile_pool(name="zp", bufs=2) as zp,
        ):
            for i in [t for _ in range(repeat) for t in range(N_TILES)]:
                xt = xp.tile([P, F], f32)
                nc.sync.dma_start(xt[:], x_d[bass.ts(i, P), :])

                hext = hp.tile([P, NB + 1], f32)
                nc.vector.memset(hext[:, 0:1], float(F))
                nc.vector.memset(hext[:, NB:NB + 1], 0.0)

                trash_v = tv.tile([P, F], bf16)
                for j in range(N_DVE):
                    nc.vector.tensor_scalar(
                        trash_v[:], xt[:], bvals[j], None,
                        Alu.is_ge, Alu.add,
                        accum_out=hext[:, 1 + j:2 + j],
                    )

                if n_act:
                    lbuf = lp.tile([P, n_act], f32)
                    for k in range(n_act):
                        j = N_DVE + k
                        sgn = sp.tile([P, F], bf16)
                        nc.scalar.activation(
                            sgn[:], xt[:], Act.Sign,
                            bias=bvals[j], scale=-1.0,
                        )
                        trash_s = ts.tile([P, F], bf16)
                        nc.scalar.activation(
                            trash_s[:], sgn[:], Act.Relu,
                            accum_out=lbuf[:, k:k + 1],
                        )
                    # H_j = F - L_j, ACT-side into an ACT-owned tile; a single
                    # DVE copy then moves it into hext. Every cross-engine
                    # handoff tile has exactly one writer instruction (more
                    # blows the per-instruction sync-wait limit in codegen).
                    hact = hp2.tile([P, n_act], f32)
                    nc.scalar.activation(
                        hact[:], lbuf[:], Act.Identity,
                        bias=float(F), scale=-1.0,
                    )
                    nc.vector.tensor_copy(hext[:, 1 + N_DVE:1 + NTH], hact[:])

                zt = zp.tile([P, NB], f32)
                nc.vector.tensor_tensor(
                    zt[:], hext[:, 0:NB], hext[:, 1:NB + 1], Alu.subtract,
                )
                nc.vector.tensor_scalar(
                    zt[:], zt[:], float(2.0 ** -10), None, Alu.mult,
                )
                nc.sync.dma_start(z_d[bass.ts(i, P), :], zt[:])

    if not nc.is_finalized():
        nc.finalize()
    return nc


def _get_program(b):
    key = b.tobytes()
    if key not in _PROGRAM_CACHE:
        _PROGRAM_CACHE[key] = _build_program([float(v) for v in b])
    return _PROGRAM_CACHE[key]


def run(x, boundaries, trace=False):
    """Run on hardware; returns (z, BassKernelResults)."""
    from concourse.bass_utils import run_bass_kernel_spmd

    x = np.ascontiguousarray(np.asarray(x), dtype=np.float32)
    b = np.ascontiguousarray(np.asarray(boundaries), dtype=np.float32)
    assert x.shape == (B, T, F) and b.shape == (NTH,)

    nc = _get_program(b)
    bpc = B // N_CORES
    in_maps = [
        {"x": np.ascontiguousarray(x[c * bpc:(c + 1) * bpc].reshape(ROWS_PER_CORE, F))}
        for c in range(N_CORES)
    ]
    res = run_bass_kernel_spmd(nc, in_maps, core_ids=list(range(N_CORES)), trace=trace)
    z = np.stack([res.results[c]["z"].reshape(bpc, T, NB) for c in range(N_CORES)])
    return z.reshape(B, T, NB), res


def kernel(x, boundaries, nr_of_bins):
    assert int(nr_of_bins) == NB
    z, _ = run(x, boundaries)
    return z
